# revision 12
# baseline (speedup 1.0000x reference)
"""AbilityEncoder TRN2 kernel v6.

One-hot bf16 encodings x host-folded bf16 tables (a bf16 moving operand
streams at full PE rate; fp8 streams at half rate on this hardware).

Leaf handling: a level-2 node is a leaf iff its operand id is 0 (~6.5%
of nodes). Instead of a dense 16-node leaf-value pass, leaves are packed
into 2 slots per parent on the host. Trees where some parent has >2 leaf
children (~0.4%) are permuted into the final "hard" tile per core, which
runs the dense 16-node leaf pass; the host inverse-permutes the output.

Output is written H-major [H, NPC] (single W2aug-stationary matmul per
tile); the host transposes.
"""
import numpy as np
import ml_dtypes
import sys

sys.path.insert(0, "/opt/trn_rl_repo")

H = 96
NODES = 21
N = 32768
NCORES = 8
NPC = N // NCORES
T = 512
NT = NPC // T
RA = 42   # encA rows
RL = 26   # encL rows
BF = ml_dtypes.bfloat16

# relu-evacuation engine split (A=scalar/ACT, D=vector/DVE)
PAT_EASY = "ADAADAADAADAADAADAAD"      # 20 evacs: 13 A, 7 D
PAT_HARD = "ADAADAADAADAADAADAADAADA"  # 24 evacs: 16 A, 8 D


def _pack_host(trigger_ids, action_ids, target_ids, operand_ids,
               trig_table, eff_table, targ_table, op_table,
               W1, b1, W2, b2, Wl, bl):
    f64 = np.float64
    W1 = W1.astype(f64); W2 = W2.astype(f64); Wl = Wl.astype(f64)
    W1top, W1bot = W1[:H], W1[H:]
    W2W1 = W2 @ W1bot

    tblA = np.zeros((RA, H), f64)
    tblA[0:7] = trig_table.astype(f64) @ W1bot + b1.astype(f64)
    tblA[7:16] = eff_table.astype(f64) @ W1bot
    tblA[16:26] = targ_table.astype(f64) @ W1bot
    tblA[26:41] = op_table.astype(f64) @ W1top
    tblA[41] = b2.astype(f64) @ W1bot
    tblB = np.zeros((RL, H), f64)
    tblB[0:7] = trig_table.astype(f64) @ Wl + bl.astype(f64)
    tblB[7:16] = eff_table.astype(f64) @ Wl
    tblB[16:26] = targ_table.astype(f64) @ Wl

    tbls = np.zeros((128, H), f64)
    tbls[0:RA] = tblA
    tbls[64:64 + RL] = tblB
    tbls[96:96 + RL] = tblB

    W2aug = np.zeros((H + 1, H), f64)
    W2aug[:H] = W2
    W2aug[H] = b2.astype(f64)
    b2w1x4 = 4.0 * (b2.astype(f64) @ W1bot)

    t = trigger_ids.astype(np.int64); a = action_ids.astype(np.int64)
    g = target_ids.astype(np.int64); o = operand_ids.astype(np.int64)
    leaf = (o == 0)                      # [N, 21]; only cols 5:21 are used
    hasb2 = bool(np.any(b2 != 0))

    # trees needing >2 leaf slots in some parent go to the hard tile
    lcnt = leaf[:, 5:21].reshape(N, 4, 4).sum(axis=2)   # [N, 4]
    hard = (lcnt > 2).any(axis=1)
    perms = []
    fallback = False
    for cix in range(NCORES):
        hd = hard[cix * NPC:(cix + 1) * NPC]
        if hd.sum() > T:
            fallback = True
        easy_ix = np.nonzero(~hd)[0]
        hard_ix = np.nonzero(hd)[0]
        perms.append(np.concatenate([easy_ix, hard_ix]))
    if fallback:
        perms = [np.arange(NPC) for _ in range(NCORES)]
    n_easy_tiles = 0 if fallback else NT - 1

    encA_c = []
    encL_c = []
    for cix in range(NCORES):
        ix = perms[cix] + cix * NPC      # global tree indices, permuted
        tc_ = t[ix]; ac = a[ix]; gc = g[ix]; oc = o[ix]
        lfc = leaf[ix]
        cols = np.arange(NPC)
        eA = np.zeros((RA, NODES, NPC), BF)
        for n in range(NODES):
            if n < 5:
                eA[:, n][tc_[:, n], cols] = 1.0
                eA[:, n][7 + ac[:, n], cols] = 1.0
                eA[:, n][16 + gc[:, n], cols] = 1.0
                eA[:, n][26 + oc[:, n], cols] = 1.0
                if hasb2 and n >= 1:
                    j = n - 1
                    cnt = (~lfc[:, 5 + 4 * j:5 + 4 * j + 4]).sum(axis=1)
                    eA[:, n][41, cols] = cnt.astype(f64)
            else:
                nl = ~lfc[:, n]
                eA[:, n][tc_[nl, n], cols[nl]] = 1.0
                eA[:, n][7 + ac[nl, n], cols[nl]] = 1.0
                eA[:, n][16 + gc[nl, n], cols[nl]] = 1.0
                eA[:, n][26 + oc[nl, n], cols[nl]] = 1.0
        # leaf encodings, 16 slot-rows:
        #   easy tree:  slot s (0,1) of parent j -> row 8*s + j
        #   hard tree:  even child c of parent j -> row 2*j + c//2
        #               odd child c -> row 8 + 2*j + c//2
        eL = np.zeros((RL, 16, NPC), BF)
        easy_n = n_easy_tiles * T
        easy_mask = cols < easy_n

        def put(row, m, n_arr):
            mm = cols[m]
            nn_ = n_arr[m] if n_arr.ndim else n_arr
            eL[:, row][tc_[mm, nn_], mm] = 1.0
            eL[:, row][7 + ac[mm, nn_], mm] = 1.0
            eL[:, row][16 + gc[mm, nn_], mm] = 1.0

        for j in range(4):
            filled = np.zeros(NPC, np.int64)
            for c in range(4):
                n = 5 + 4 * j + c
                is_lf = lfc[:, n]
                e0 = is_lf & easy_mask & (filled == 0)
                e1 = is_lf & easy_mask & (filled == 1)
                put(j, e0, np.full(NPC, n))
                put(8 + j, e1, np.full(NPC, n))
                hd_m = is_lf & ~easy_mask
                if c % 2 == 0:
                    put(2 * j + c // 2, hd_m, np.full(NPC, n))
                else:
                    put(8 + 2 * j + c // 2, hd_m, np.full(NPC, n))
                filled = filled + (is_lf & easy_mask).astype(np.int64)
        encA_c.append(eA)
        encL_c.append(eL)

    return (encA_c, encL_c, perms, n_easy_tiles, tbls.astype(BF),
            W2W1.astype(BF), W1bot.astype(BF), W2aug.astype(BF),
            b2w1x4.astype(np.float32).reshape(H, 1))


_CACHED = {}
_SKIP_SPLIT = False


def _patch_tile(tile, mybir, _br, ScopedClock):
    def _drain_and_barrier(self, tick_clock, wait_clock):
        nc_ = self.nc
        probe = nc_.sync.drain()
        wait_clock.add_sem_waits(probe.ins,
                                 ScopedClock({None: tick_clock.global_clock}))
        si = probe.ins.sync_info
        waits = list(si.on_wait) if si is not None else []
        if len(waits) > 1:
            si.on_wait = waits[:1]
            for w in waits[1:]:
                extra = nc_.sync.drain()
                extra.ins.sync_info = _br.SyncInfo(on_wait=[w], on_update=[])
        nc_.all_engine_barrier()
        popped = nc_._tile_sem_poison_stack.pop()
        assert popped is self._sem_poison
        nc_.clear_and_free_semaphores(list(self.sems.allocated().values()))
        nc_.all_engine_barrier()

    tile.TileContext._drain_and_barrier = _drain_and_barrier


def _split_waits(nc_, mybir, _br, max_waits=1):
    for f in nc_.m.functions:
        for bb in f.blocks:
            out = []
            for inst in bb.instructions:
                si = inst.sync_info
                if si is not None:
                    waits = list(si.on_wait)
                    if len(waits) > max_waits:
                        extra, keep = waits[:-max_waits], waits[-max_waits:]
                        for j, w in enumerate(extra):
                            ev = mybir.InstEventSemaphore(
                                name=f"{inst.name}-xw{j}")
                            ev.engine = inst.engine
                            ev.sync_info = _br.SyncInfo(
                                on_wait=[w], on_update=[])
                            out.append(ev)
                        si.on_wait = keep
                out.append(inst)
            try:
                bb.instructions = out
            except Exception:
                bb.instructions.clear()
                for i_ in out:
                    bb.instructions.append(i_)


def _build_program(n_easy_tiles):
    import concourse.bass as bass
    import concourse.tile as tile
    import concourse.mybir as mybir
    from concourse.vector_clock import ScopedClock
    import bass_rust as _br

    _patch_tile(tile, mybir, _br, ScopedClock)

    dt = mybir.dt
    Relu = mybir.ActivationFunctionType.Relu
    ADD = mybir.AluOpType.add
    nc = bass.Bass(trn_type="TRN2", target_bir_lowering=False, debug=False)
    encA_d = nc.dram_tensor("encA", [RA, NT * NODES * T], dt.bfloat16,
                            kind="ExternalInput").ap()
    encL_d = nc.dram_tensor("encL", [RL, NT * 16 * T], dt.bfloat16,
                            kind="ExternalInput").ap()
    tbls_d = nc.dram_tensor("tbls", [128, H], dt.bfloat16,
                            kind="ExternalInput").ap()
    w2w1_d = nc.dram_tensor("w2w1", [H, H], dt.bfloat16,
                            kind="ExternalInput").ap()
    w1bot_d = nc.dram_tensor("w1bot", [H, H], dt.bfloat16,
                             kind="ExternalInput").ap()
    w2aug_d = nc.dram_tensor("w2aug", [H + 1, H], dt.bfloat16,
                             kind="ExternalInput").ap()
    b2c_d = nc.dram_tensor("b2c", [H, 1], dt.float32,
                           kind="ExternalInput").ap()
    out_d = nc.dram_tensor("out", [H, NPC], dt.float32,
                           kind="ExternalOutput").ap()

    with tile.TileContext(nc) as tc:
        with tc.tile_pool(name="const", bufs=1) as cpool, \
             tc.tile_pool(name="enc", bufs=3) as epool, \
             tc.tile_pool(name="hbuf", bufs=4) as hpool, \
             tc.tile_pool(name="hb2", bufs=8) as h2pool, \
             tc.tile_pool(name="hb3", bufs=3) as h3pool, \
             tc.tile_pool(name="ps", bufs=3, space="PSUM") as pspool, \
             tc.tile_pool(name="psb", bufs=1, space="PSUM") as psbpool, \
             tc.tile_pool(name="p1", bufs=2, space="PSUM") as p1pool, \
             tc.tile_pool(name="p0", bufs=1, space="PSUM") as p0pool:

            tbls_s = cpool.tile([128, H], dt.bfloat16)
            nc.sync.dma_start(tbls_s[:], tbls_d[:])
            w2w1_s = cpool.tile([H, H], dt.bfloat16)
            nc.sync.dma_start(w2w1_s[:], w2w1_d[:])
            w1bot_s = cpool.tile([H, H], dt.bfloat16)
            nc.sync.dma_start(w1bot_s[:], w1bot_d[:])
            w2aug_s = cpool.tile([H + 1, H], dt.bfloat16)
            nc.sync.dma_start(w2aug_s[:], w2aug_d[:])
            b2c_s = cpool.tile([H, 1], dt.float32)
            nc.sync.dma_start(b2c_s[:], b2c_d[:])

            tbA = tbls_s[0:RA, :]
            tbBe = tbls_s[64:64 + RL, :]
            tbBo = tbls_s[96:96 + RL, :]

            st = {}   # per-tile pipeline state

            def stage1_lvl2(it):
                easy = it < n_easy_tiles
                nslot = 4 if easy else 8
                enc = epool.tile([128, NODES * T], dt.bfloat16, tag="enc")
                nc.sync.dma_start(
                    enc[0:RA, :],
                    encA_d[:, it * NODES * T:(it + 1) * NODES * T])
                base = it * 16 * T
                nc.sync.dma_start(
                    enc[64:64 + RL, 0:nslot * T],
                    encL_d[:, base:base + nslot * T])
                nc.sync.dma_start(
                    enc[96:96 + RL, 0:nslot * T],
                    encL_d[:, base + 8 * T:base + (8 + nslot) * T])

                def eA(n):
                    return enc[0:RA, n * T:(n + 1) * T]

                def eS(row, odd_copy):
                    r0 = 96 if odd_copy else 64
                    return enc[r0:r0 + RL, row * T:(row + 1) * T]

                nrelu = 0
                pat = PAT_EASY if easy else PAT_HARD

                def evac(dst, src):
                    nonlocal nrelu
                    if pat[nrelu] == "A":
                        nc.scalar.activation(dst, src, Relu)
                    else:
                        nc.vector.tensor_scalar_max(dst, src, 0.0)
                    nrelu += 1

                hA = []
                hB = []
                for j in range(4):
                    hA_j = hpool.tile([H, 4, T], dt.bfloat16, tag="hA")
                    hA.append(hA_j)
                    hB_j = hpool.tile([H, 4, T], dt.bfloat16, tag="hB")
                    hB.append(hB_j)
                    n0 = 5 + 4 * j
                    pa = []
                    for c in range(4):
                        pa_c = pspool.tile([128, T], dt.float32, tag="ps",
                                           name=f"pa{c}")
                        pa.append(pa_c)
                        nc.tensor.matmul(pa_c[0:H, :], tbA, eA(n0 + c),
                                         start=True, stop=True)
                        if easy:
                            if c == 0:
                                psB = psbpool.tile([128, 2 * T], dt.float32,
                                                   tag="psb")
                                nc.tensor.matmul(psB[0:H, 0:T], tbBe,
                                                 eS(j, False),
                                                 start=True, stop=True,
                                                 tile_position=(64, 0))
                                nc.tensor.matmul(psB[0:H, T:2 * T], tbBo,
                                                 eS(j, True),
                                                 start=True, stop=True,
                                                 tile_position=(96, 0))
                        else:
                            if c == 0:
                                psB1 = psbpool.tile([128, 2 * T], dt.float32,
                                                    tag="psb")
                                nc.tensor.matmul(psB1[0:H, 0:T], tbBe,
                                                 eS(2 * j, False),
                                                 start=True, stop=True,
                                                 tile_position=(64, 0))
                                nc.tensor.matmul(psB1[0:H, T:2 * T], tbBo,
                                                 eS(2 * j, True),
                                                 start=True, stop=True,
                                                 tile_position=(96, 0))
                            if c == 2:
                                psB2 = psbpool.tile([128, 2 * T], dt.float32,
                                                    tag="psb")
                                nc.tensor.matmul(psB2[0:H, 0:T], tbBe,
                                                 eS(2 * j + 1, False),
                                                 start=True, stop=True,
                                                 tile_position=(64, 0))
                                nc.tensor.matmul(psB2[0:H, T:2 * T], tbBo,
                                                 eS(2 * j + 1, True),
                                                 start=True, stop=True,
                                                 tile_position=(96, 0))
                        if c >= 1:
                            evac(hA_j[:, c - 1:c, :], pa[c - 1][0:H, :])
                        if c == 1 and not easy:
                            evac(hB_j[:, 0:2, :], psB1[0:H, :])
                        if c == 2 and easy:
                            evac(hB_j[:, 0:2, :], psB[0:H, :])
                    evac(hA_j[:, 3:4, :], pa[3][0:H, :])
                    if not easy:
                        evac(hB_j[:, 2:4, :], psB2[0:H, :])

                # sibling sums
                sh = []
                sl = []
                for j in range(4):
                    sh_j = h2pool.tile([H, T], dt.bfloat16, tag="sh")
                    sl_j = h2pool.tile([H, T], dt.bfloat16, tag="sl")
                    sh.append(sh_j)
                    sl.append(sl_j)
                    tmpA = hpool.tile([H, 2, T], dt.bfloat16, tag="tmpA")
                    nc.vector.tensor_tensor(out=tmpA[:], in0=hA[j][:, 0:2, :],
                                            in1=hA[j][:, 2:4, :], op=ADD)
                    nc.vector.tensor_tensor(out=sh_j[:], in0=tmpA[:, 0, :],
                                            in1=tmpA[:, 1, :], op=ADD)
                    if easy:
                        nc.vector.tensor_tensor(out=sl_j[:],
                                                in0=hB[j][:, 0, :],
                                                in1=hB[j][:, 1, :], op=ADD)
                    else:
                        tmpB = hpool.tile([H, 2, T], dt.bfloat16, tag="tmpB")
                        nc.vector.tensor_tensor(out=tmpB[:],
                                                in0=hB[j][:, 0:2, :],
                                                in1=hB[j][:, 2:4, :], op=ADD)
                        nc.vector.tensor_tensor(out=sl_j[:],
                                                in0=tmpB[:, 0, :],
                                                in1=tmpB[:, 1, :], op=ADD)
                st[it] = {"enc": enc, "sh": sh, "sl": sl}

            def stage2_lvl1(it):
                d = st[it]
                enc = d["enc"]
                sh = d["sh"]
                sl = d["sl"]

                def eA(n):
                    return enc[0:RA, n * T:(n + 1) * T]

                h1 = hpool.tile([H, 4, T], dt.bfloat16, tag="h1")
                for pair in range(2):
                    ps = [p1pool.tile([128, T], dt.float32, tag="p1",
                                      name=f"ps1_{pair}_{q_}")
                          for q_ in range(2)]
                    js = (2 * pair, 2 * pair + 1)
                    for q, j in enumerate(js):
                        nc.tensor.matmul(ps[q][0:H, :], tbA, eA(1 + j),
                                         start=True, stop=False)
                    for q, j in enumerate(js):
                        nc.tensor.matmul(ps[q][0:H, :], w2w1_s[:], sh[j][:],
                                         start=False, stop=False)
                    for q, j in enumerate(js):
                        nc.tensor.matmul(ps[q][0:H, :], w1bot_s[:], sl[j][:],
                                         start=False, stop=True)
                    for q, j in enumerate(js):
                        nc.scalar.activation(h1[:, j:j + 1, :],
                                             ps[q][0:H, :], Relu)
                # root child sum
                s0 = h3pool.tile([H, T], dt.bfloat16, tag="s0")
                tmp0 = hpool.tile([H, 2, T], dt.bfloat16, tag="tmp0")
                nc.vector.tensor_tensor(out=tmp0[:], in0=h1[:, 0:2, :],
                                        in1=h1[:, 2:4, :], op=ADD)
                nc.vector.tensor_tensor(out=s0[:], in0=tmp0[:, 0, :],
                                        in1=tmp0[:, 1, :], op=ADD)
                d["s0"] = s0

            def stage3_root(it):
                d = st[it]
                enc = d["enc"]
                ps0 = p0pool.tile([128, T], dt.float32, tag="p0")
                nc.tensor.matmul(ps0[0:H, :], tbA, enc[0:RA, 0:T],
                                 start=True, stop=False)
                nc.tensor.matmul(ps0[0:H, :], w2w1_s[:], d["s0"][:],
                                 start=False, stop=True)
                h0 = h3pool.tile([H + 1, T], dt.bfloat16, tag="h0")
                nc.vector.memset(h0[H:H + 1, :], 1.0)
                nc.scalar.activation(h0[0:H, :], ps0[0:H, :], Relu,
                                     bias=b2c_s[:])
                d["h0"] = h0

            def stage4_out(it):
                d = st.pop(it)
                pso = p0pool.tile([128, T], dt.float32, tag="p0")
                nc.tensor.matmul(pso[0:H, :], w2aug_s[:], d["h0"][:],
                                 start=True, stop=True)
                osb = hpool.tile([H, T], dt.float32, tag="osb")
                if it % 2 == 0:
                    nc.scalar.copy(osb[:], pso[0:H, :])
                else:
                    nc.vector.tensor_copy(out=osb[:], in_=pso[0:H, :])
                nc.gpsimd.dma_start(out_d[:, it * T:(it + 1) * T], osb[:])

            for k in range(NT + 3):
                if k < NT:
                    stage1_lvl2(k)
                if 0 <= k - 3 < NT:
                    stage4_out(k - 3)
                if 0 <= k - 1 < NT:
                    stage2_lvl1(k - 1)
                if 0 <= k - 2 < NT:
                    stage3_root(k - 2)

    if not _SKIP_SPLIT:
        _split_waits(nc, mybir, _br)
    return nc


def _make_in_maps(host):
    (encA_c, encL_c, perms, n_easy, tbls, W2W1, W1bot, W2aug, b2c) = host
    in_maps = []
    for cix in range(NCORES):
        def tilefmt(e, nn):
            a = e.reshape(e.shape[0], nn, NT, T).transpose(0, 2, 1, 3)
            return np.ascontiguousarray(a.reshape(e.shape[0], NT * nn * T))

        in_maps.append({
            "encA": tilefmt(encA_c[cix], NODES),
            "encL": tilefmt(encL_c[cix], 16),
            "tbls": tbls, "w2w1": W2W1, "w1bot": W1bot,
            "w2aug": W2aug, "b2c": b2c,
        })
    return in_maps


def kernel(**inputs) -> np.ndarray:
    from concourse.bass_utils import run_bass_kernel_spmd

    host = _pack_host(**inputs)
    n_easy = host[3]
    if _CACHED.get("n_easy") != n_easy:
        _CACHED["nc"] = _build_program(n_easy)
        _CACHED["n_easy"] = n_easy
    nc = _CACHED["nc"]
    perms = host[2]
    in_maps = _make_in_maps(host)
    res = run_bass_kernel_spmd(nc, in_maps, list(range(NCORES)))
    outs = []
    for c in range(NCORES):
        o = res.results[c]["out"].T          # [NPC, H], permuted tree order
        inv = np.empty(NPC, np.int64)
        inv[perms[c]] = np.arange(NPC)
        outs.append(o[inv])
    return np.ascontiguousarray(np.concatenate(outs, axis=0)).astype(
        np.float32)
